# revision 8
# baseline (speedup 1.0000x reference)
"""Sharded shift-invariant KNN retrieval on 8 Trainium2 NeuronCores.

Math (matches the reference nn.Module):
  - Query snapshots are one-hot gathers of the input sequence, so each
    normalized query vector has entries in {0, 1/8} exactly (64 ones out of
    C*L2 = 8192 slots, norm exactly 8). The 1/8 and the database norms are
    folded into a per-database-row scale applied on the host, so the device
    computes a pure A @ D^T with A in {0,1} (exact in bf16) and D prescaled
    bf16: only the database rounding (~2^-9 relative) enters the result.
  - The 5 circular shifts are materialized as extra query rows (M = 5*64 =
    320, m = s*64 + bq); the device takes the max over the 5 shift blocks.
  - The database (N=20000) is sharded across the 8 cores (2500 rows each,
    padded to 2560); each core computes its (64, 2560) slice of sim; the
    host gathers the slices and derives top_cls, with an exact fp64 rescore
    of each query's device top-k to pin the argmax.

Device kernel per core (PE-bound, ~160us steady-state):
  - Database rows sit on PSUM partitions: stationary weights are 128x128
    chunks of D^T, the moving operand is the 320-column query matrix A^T.
    Every matmul streams exactly the 320 useful columns (100% packing:
    64 k-chunks x 20 n-chunks = 1280 MMs x 320 cols = 409.6K PE cycles,
    the dense-formulation floor; measured at the per-MM roofline).
  - The shift-max over the 5 m-segments of 64 is pure free-dim slicing on
    the vector engine (one PSUM->SBUF copy per n-chunk; the hardware
    forbids two PSUM operands in one vector op).
  - Output is written transposed, (NPAD, 64) per core; the host reassembles.
"""

import numpy as np
import ml_dtypes

B, L1, C = 8, 2048, 128
Q, L2 = 8, 64
N = 20000
SHIFT = 2
EPS = 1e-8
NCORES = 8
NSHARD = N // NCORES          # 2500
P = 128
KO = (C * L2) // P            # 64 k-chunks of 128
KOG = 4                       # k-chunk groups (DMA granularity)
KPG = KO // KOG               # 16
M = (2 * SHIFT + 1) * B * Q   # 320 query rows (5 shifts x 64 queries)
NT = 512
NTILES = 5
NPAD = NT * NTILES            # 2560 padded shard columns

_bf16 = ml_dtypes.bfloat16
_NC_CACHE = {}


# ---------------------------------------------------------------- device ----

def _build_nc(repeat=1):
    from contextlib import ExitStack

    import concourse.tile as tile
    from concourse import bacc, mybir

    nc = bacc.Bacc("TRN2", target_bir_lowering=False, debug=False,
                   num_devices=NCORES)
    at4 = nc.dram_tensor("at", [KOG, P, KPG, M], mybir.dt.bfloat16,
                         kind="ExternalInput").ap()
    dt_raw = nc.dram_tensor("dt", [KO * P, NPAD], mybir.dt.bfloat16,
                            kind="ExternalInput").ap()
    simt = nc.dram_tensor("sim", [NPAD, 64], mybir.dt.float32,
                          kind="ExternalOutput").ap()
    dt = dt_raw.rearrange("(ko p) n -> p ko n", p=P)

    bf = mybir.dt.bfloat16
    f32 = mybir.dt.float32
    mx = mybir.AluOpType.max
    NC4 = NT // P

    with tile.TileContext(nc) as tc, ExitStack() as ctx:
      a_pool = ctx.enter_context(tc.tile_pool(name="a", bufs=1))
      d_pool = ctx.enter_context(tc.tile_pool(name="d", bufs=4))
      o_pool = ctx.enter_context(tc.tile_pool(name="o", bufs=3))
      ps_pool = ctx.enter_context(tc.tile_pool(name="ps", bufs=2,
                                               space="PSUM"))
      for _rep in range(repeat):
        a_ch = []
        for g in range(KOG):
            a_sb = a_pool.tile([P, KPG, M], bf, tag=f"a{g}", name=f"a{g}_{_rep}")
            nc.scalar.dma_start(a_sb[:], at4[g])
            a_ch.append(a_sb)

        for t in range(NTILES):
            pss = [ps_pool.tile([P, M], f32, tag=f"ps{j}", name=f"ps{j}_{t}_{_rep}")
                   for j in range(NC4)]
            for g in range(KOG):
                d_sb = d_pool.tile([P, KPG, NT], bf, tag="dslab",
                                   name=f"dslab_{t}_{g}_{_rep}")
                nc.sync.dma_start(
                    d_sb[:],
                    dt[:, g * KPG:(g + 1) * KPG, t * NT:(t + 1) * NT])
                for k in range(KPG):
                    ko = g * KPG + k
                    st = ko == 0
                    sp = ko == KO - 1
                    rhs = a_ch[g][:, k, :]
                    for j in range(NC4):
                        nc.tensor.matmul(pss[j][:],
                                         d_sb[:, k, j * P:(j + 1) * P],
                                         rhs, start=st, stop=sp)
            out = o_pool.tile([P, NC4, 64], f32, tag="out", name=f"out_{t}_{_rep}")
            for j in range(NC4):
                # DVE can't take two PSUM operands in one op: copy to SBUF
                # first, then the shift-max is pure free-dim slicing.
                sb = o_pool.tile([P, M], f32, tag="sb", name=f"sb_{t}_{j}_{_rep}")
                nc.vector.tensor_copy(sb[:], pss[j][:])
                m1 = o_pool.tile([P, 64], f32, tag="m1", name=f"m1_{t}_{j}_{_rep}")
                m2 = o_pool.tile([P, 64], f32, tag="m2", name=f"m2_{t}_{j}_{_rep}")
                nc.vector.tensor_tensor(m1[:], sb[:, 0:64], sb[:, 64:128], mx)
                nc.vector.tensor_tensor(m2[:], sb[:, 128:192], sb[:, 192:256], mx)
                nc.vector.tensor_tensor(m1[:], m1[:], sb[:, 256:320], mx)
                nc.vector.tensor_tensor(out[:, j, :], m1[:], m2[:], mx)
            # simt rows [t*NT, (t+1)*NT) as (P, NC4, 64): row = t*NT + j*P + p
            nc.scalar.dma_start(
                simt[t * NT:(t + 1) * NT, :].rearrange("(j p) m -> p j m", p=P),
                out[:])

    nc.compile()
    return nc


def _get_nc():
    if "nc" not in _NC_CACHE:
        _NC_CACHE["nc"] = _build_nc()
    return _NC_CACHE["nc"]


# ------------------------------------------------------------ host prep ----

def _snapshot_channels(seq_input, kp_starts, kp_lens):
    """(B*Q, L2) int channel index per snapshot pixel."""
    seq = np.asarray(seq_input)
    ks = np.asarray(kp_starts).astype(np.int64)
    kl = np.asarray(kp_lens).astype(np.int64)
    lens = np.maximum(kl, 1)
    pos = ks[..., None] + (lens[..., None] * np.arange(L2, dtype=np.int64)) // L2
    pos = np.clip(pos, 0, L1 - 1)
    ch = np.take_along_axis(
        np.broadcast_to(seq[:, None, :], (B, Q, L1)), pos, axis=2
    )
    return ch.reshape(B * Q, L2)


def _build_queries(ch):
    """A^T tiled (KOG, P, KPG, M) bf16 with entries {0,1}; m = s*64 + bq."""
    sn = np.zeros((B * Q, C, L2), np.float32)
    bq = np.repeat(np.arange(B * Q), L2)
    ll = np.tile(np.arange(L2), B * Q)
    sn[bq, ch.reshape(-1), ll] = 1.0
    A = np.zeros((2 * SHIFT + 1, B * Q, C, L2), np.float32)
    for si in range(2 * SHIFT + 1):
        A[si] = np.roll(sn, si - SHIFT, axis=-1)
    A = A.reshape(M, C * L2)
    # at4[g, p, k, m] = A[m, (g*KPG + k)*P + p]
    return np.ascontiguousarray(
        A.T.reshape(KOG, KPG, P, M).transpose(0, 2, 1, 3)
    ).astype(_bf16)


def _db_norms(db2d):
    """fp64 row norms, chunked to bound memory."""
    out = np.empty(N, np.float64)
    step = 2048
    for i in range(0, N, step):
        blk = db2d[i:i + step].astype(np.float64)
        out[i:i + step] = np.sqrt(np.einsum("nk,nk->n", blk, blk))
    return out


def _build_db_shards(db2d, scale):
    """Per-core (KO*P, NPAD) bf16 transposed, prescaled shards."""
    shards = []
    for c in range(NCORES):
        d = (db2d[c * NSHARD:(c + 1) * NSHARD]
             * scale[c * NSHARD:(c + 1) * NSHARD, None]).astype(_bf16)
        dt = np.zeros((KO * P, NPAD), _bf16)
        dt[:, :NSHARD] = d.T
        shards.append(dt)
    return shards


# ------------------------------------------------------------- top_cls ----

def _exact_topcls(sim, ch, db2d, scale64, db_classes, k=16):
    """argmax per query, pinned by an exact fp64 rescore of the device
    top-k candidates."""
    sp = np.arange(-SHIFT, SHIFT + 1)
    jj = np.arange(L2)
    cols = (jj[None, :] + sp[:, None]) % L2          # (5, L2)
    top = np.argpartition(sim, N - k, axis=1)[:, N - k:]  # (64, k)
    best = np.empty(B * Q, np.int64)
    for i in range(B * Q):
        rows = top[i]
        d = db2d[rows].reshape(k, C, L2).astype(np.float64)
        d *= scale64[rows][:, None, None]
        d2 = d[:, ch[i], :]                          # (k, L2j, L2l)
        smax = np.full(k, -np.inf)
        for s in range(2 * SHIFT + 1):
            v = d2[:, jj, cols[s]].sum(axis=1)       # (k,)
            smax = np.maximum(smax, v)
        # first-index tie-break to match reference argmax semantics
        order = np.lexsort((rows, -smax))
        best[i] = rows[order[0]]
    return np.asarray(db_classes)[best]


# -------------------------------------------------------------- kernel ----

def kernel(seq_input, kp_starts, kp_lens, database, db_classes):
    db_classes = np.asarray(db_classes)
    db2d = np.ascontiguousarray(np.asarray(database).reshape(N, C * L2))

    ch = _snapshot_channels(seq_input, kp_starts, kp_lens)
    at4 = _build_queries(ch)
    norm64 = _db_norms(db2d)
    scale = (1.0 / (8.0 * (norm64 + EPS))).astype(np.float32)
    shards = _build_db_shards(db2d, scale)

    nc = _get_nc()
    from concourse.bass_utils import run_bass_kernel_spmd
    in_maps = [{"at": at4, "dt": shards[c]} for c in range(NCORES)]
    res = run_bass_kernel_spmd(nc, in_maps, list(range(NCORES))).results

    sim = np.concatenate(
        [res[c]["sim"][:NSHARD, :].T for c in range(NCORES)], axis=1
    )                                                # (64, 20000) f32

    scale64 = 1.0 / (8.0 * (norm64 + EPS))
    top_cls = _exact_topcls(sim, ch, db2d, scale64, db_classes)

    return (
        sim.reshape(B, Q, N).astype(np.float32),
        top_cls.reshape(B, Q).astype(db_classes.dtype),
    )
